# revision 1
# baseline (speedup 1.0000x reference)
"""Trainium2 Bass kernel for nn_AudioModel (LSTM(40->64) -> last-h -> MLP head).

Strategy (8 NeuronCores, pure data parallel, no collectives):
  - Each core processes a 1024-row batch shard, batch split into halves
    A/B stacked on SBUF partitions (rows 0-63 = A units, 64-127 = B units)
    so every op uses all 128 lanes; free dim = 512 batch columns per half.
  - The 512 free columns are split into NS=3 independent column streams
    that pipeline the serial time recurrence against each other.  Streams
    rotate through engines (anti-phased via an initial skew + PSUM
    round-robin), so ScalarE / VectorE / GpSimd / PE all stay busy.
  - Per stream and timestep:
      PE:      4 x-side matmuls (K=82: feats+bias rows for A|B block-diag)
               accumulate with 4 h-side matmuls (K=128 block-diag) in PSUM.
      ScalarE: ONE sigmoid over all 4 gate banks ([i,f,o,g]; bank g is
               sigmoid(2a_g) so tanh(g) = 2sig-1 folds into the cell stt).
      VectorE: t1 = (sig_g-.5)*sig_i (stt); c~ = t1 + fm (tensor_add);
               h' = LSTM_TAU_MUL custom op  = c~*sig_o*(a_u + u*(b_u+u*cg)),
               u = min(c~^2, 1)  -- a per-partition-coefficient degree-5
               odd polynomial approximating tanh(2c~)/2, coefficients fit
               per hidden unit against the empirical c~ distribution from a
               host-side calibration run (rel-err contribution ~3e-3).
      GpSimd:  fm = sig_f * c~_prev  (keeps VectorE off the critical path).
  - The cell is tracked as c~ = c/2 and h as h' = h/2 (weights pre-scaled),
    which makes every pointwise op a plain fused ALU op.
  - feats are transposed on the host into [T, 82, 512] fp16 tiles so all
    device DMAs are contiguous; biases ride constant-1 feature rows.
"""

import os
import sys
from contextlib import ExitStack, nullcontext

import numpy as np

_BF = np.dtype(np.float16)

for _p in ("/opt/trn_rl_repo",):
    if _p not in sys.path:
        sys.path.insert(0, _p)

import concourse.bass as bass
import concourse.mybir as mybir
from concourse import bacc
from concourse.bass_utils import run_bass_kernel_spmd
from concourse.tile import TileContext


def _install_ntff_hook():
    """Provide antenv.axon_hooks if the image lacks it, so trace=True works."""
    try:
        import antenv.axon_hooks  # noqa: F401

        return
    except ImportError:
        pass
    import contextlib
    import ctypes
    import types

    so_path = "/opt/axon/libaxon_pjrt.so"
    hook = None
    if os.path.exists(so_path):
        try:
            lib = ctypes.CDLL(so_path)
            if hasattr(lib, "axon_start_nrt_profile"):
                lib.axon_start_nrt_profile.argtypes = [
                    ctypes.POINTER(ctypes.c_int64),
                    ctypes.c_size_t,
                ]
                lib.axon_start_nrt_profile.restype = ctypes.c_int64
                lib.axon_stop_nrt_profile.argtypes = [ctypes.c_char_p]
                lib.axon_stop_nrt_profile.restype = ctypes.c_int64

                @contextlib.contextmanager
                def _hook(output_dir, device_ids):
                    import jax

                    jax.devices()
                    if device_ids:
                        ids = (ctypes.c_int64 * len(device_ids))(*device_ids)
                        rc = lib.axon_start_nrt_profile(ids, len(device_ids))
                    else:
                        rc = lib.axon_start_nrt_profile(None, 0)
                    if rc != 0:
                        raise RuntimeError(f"axon_start_nrt_profile rc={rc}")
                    try:
                        yield
                    finally:
                        n = lib.axon_stop_nrt_profile(str(output_dir).encode())
                        print(f"profile: {n} file(s) written to {output_dir}", file=sys.stderr)

                hook = _hook
        except OSError:
            hook = None

    mod = types.ModuleType("antenv.axon_hooks")
    mod._hook = hook
    mod.get_axon_ntff_profile_hook = lambda: mod._hook
    mod.set_axon_ntff_profile_hook = lambda h: setattr(mod, "_hook", h)
    sys.modules["antenv.axon_hooks"] = mod


_install_ntff_hook()

F32 = mybir.dt.float32
FH = mybir.dt.float16
AF = mybir.ActivationFunctionType
OP = mybir.AluOpType

B, T, NI, H = 8192, 100, 40, 64
NCORES = 8
BL = B // NCORES  # 1024 rows per core
HB = BL // 2  # 512 = half-batch (free dim of all tiles)
KX = 2 * (NI + 1)  # 82 = A feats(40) + ones(1) + B feats(40) + ones(1)
NS = 3
CHUNK = [(0, 171), (171, 342), (342, 512)]  # stream column ranges
SW = 171  # max stream width (tile stride)

LAST_RESULT = None
_NC_CACHE = {}
_TAU_OP = None


def _get_tau_op():
    """Register the fused tanh-cell custom DVE op (documented authoring path:
    DveOp + OPS append; uops_sha computed from lower() at registration)."""
    global _TAU_OP
    if _TAU_OP is not None:
        return _TAU_OP
    from concourse.dve_spec import Spec, Src0, Src1, C0, C1, C2, One, sq, minn, lower
    from concourse.dve_uop import DveOpSpec
    import concourse.dve_ops as dve_ops
    from concourse.dve_ops import DveOp, OPS

    name = "LSTM_TAU_MUL"
    u = minn(sq(Src0), One)

    def _ref(in0, in1, s0, s1, imm2):
        uu = np.minimum(in0.astype(np.float32) ** 2, 1.0)
        return (in0 * in1 * (s0 + uu * (s1 + uu * imm2))).astype(np.float32)

    spec = Spec(body=(Src0 * Src1) * (C0 + u * (C1 + u * C2)), reference=_ref)
    row = dve_ops._CUSTOM_DVE_ROW_BASE + len(OPS)
    shas = {
        v: DveOpSpec(name=name, opcode=row, uops=lower(spec, ver=v), rd1_en=True).sha(v)
        for v in ("v3", "v4")
    }
    op = DveOp(name, spec, subdim=False, uops_sha=shas)
    OPS.append(op)
    dve_ops._SUB_OPCODE_FOR_NAME[name] = row
    dve_ops.CUSTOM_DVE_SPECS[name] = spec
    _TAU_OP = op
    return op


def _build_nc(cg: float):
    tau_op = _get_tau_op()
    nc = bacc.Bacc("TRN2", target_bir_lowering=False, debug=False)

    ft = nc.dram_tensor("ft", [T, KX, HB], FH, kind="ExternalInput")
    wx = nc.dram_tensor("wx", [KX, 512], FH, kind="ExternalInput")
    wh = nc.dram_tensor("wh", [128, 512], FH, kind="ExternalInput")
    w1 = nc.dram_tensor("w1", [128, 64], FH, kind="ExternalInput")
    b1 = nc.dram_tensor("b1", [64, 1], F32, kind="ExternalInput")
    w2 = nc.dram_tensor("w2", [64, 2], FH, kind="ExternalInput")
    b2 = nc.dram_tensor("b2", [2, 1], F32, kind="ExternalInput")
    ca = nc.dram_tensor("ca", [128, 1], F32, kind="ExternalInput")
    cb = nc.dram_tensor("cb", [128, 1], F32, kind="ExternalInput")
    out = nc.dram_tensor("out", [2, HB], F32, kind="ExternalOutput")

    with TileContext(nc) as tc, ExitStack() as ctx:
        const = ctx.enter_context(tc.tile_pool(name="const", bufs=1))
        ftp = ctx.enter_context(tc.tile_pool(name="ftp", bufs=16))
        gp = ctx.enter_context(tc.tile_pool(name="gp", bufs=4, space="PSUM"))
        sp = ctx.enter_context(tc.tile_pool(name="sp", bufs=4))
        dp = ctx.enter_context(tc.tile_pool(name="dp", bufs=4))
        cp = ctx.enter_context(tc.tile_pool(name="cp", bufs=3))
        hp = ctx.enter_context(tc.tile_pool(name="hp", bufs=4))

        wx_s = const.tile([KX, 512], FH)
        nc.sync.dma_start(wx_s[:], wx[:, :])
        wh_s = const.tile([128, 512], FH)
        nc.sync.dma_start(wh_s[:], wh[:, :])
        w1_s = const.tile([128, 64], FH)
        nc.sync.dma_start(w1_s[:], w1[:, :])
        b1_s = const.tile([64, 1], F32)
        nc.sync.dma_start(b1_s[:], b1[:, :])
        w2_s = const.tile([64, 2], FH)
        nc.sync.dma_start(w2_s[:], w2[:, :])
        b2_s = const.tile([2, 1], F32)
        nc.sync.dma_start(b2_s[:], b2[:, :])
        ca_s = const.tile([128, 1], F32)
        nc.sync.dma_start(ca_s[:], ca[:, :])
        cb_s = const.tile([128, 1], F32)
        nc.sync.dma_start(cb_s[:], cb[:, :])

        h_final = const.tile([128, HB], FH)

        h_prev = [None] * NS
        c_prev = [None] * NS

        for t in range(T):
            ft_t = ftp.tile([KX, HB], FH)
            nc.sync.dma_start(ft_t[:], ft[t])

            for s in range(NS):
                c0, c1 = CHUNK[s]
                F = c1 - c0
                skew = tc.tile_wait_until(0.00132 * s, enable=(t == 0)) if t == 0 else nullcontext()
                with skew:
                    # gate PSUM tile: 4 gates at 256-f32 (1KB) stride = 2 banks
                    # 2-bank PSUM tile; gates 0,1 share bank 0 and gates 2,3
                    # share bank 1.  start=True clears has_written for the
                    # WHOLE bank, so only the first matmul into each bank may
                    # set it; the second gate's x-matmul runs with flags=0,
                    # which overwrites where the bit is unset (its region)
                    # and the h-matmuls then accumulate where it is set.
                    g = gp.tile([128, 4 * 256], F32, tag="g", name=f"g{s}_{t}")
                    for X in range(4):
                        nc.tensor.matmul(
                            g[:, 256 * X : 256 * X + F],
                            wx_s[:, 128 * X : 128 * (X + 1)],
                            ft_t[:, c0:c1],
                            start=(X % 2 == 0),
                            stop=(h_prev[s] is None),
                            skip_group_check=(X % 2 == 1),
                        )
                    if h_prev[s] is not None:
                        for X in range(4):
                            nc.tensor.matmul(
                                g[:, 256 * X : 256 * X + F],
                                wh_s[:, 128 * X : 128 * (X + 1)],
                                h_prev[s],
                                start=False,
                                stop=True,
                                skip_group_check=(X % 2 == 1),
                            )

                    # one sigmoid over all 4 banks; bank order [i, f, o, g2]
                    S = sp.tile([128, 4 * SW], FH, tag=f"S{s}")
                    gv = g[:, :].rearrange("p (g c) -> p g c", c=256)[:, :, 0:F]
                    sv = S[:, :].rearrange("p (g c) -> p g c", c=SW)[:, :, 0:F]
                    nc.scalar.activation(sv, gv, AF.Sigmoid)
                    sig_i = S[:, 0 * SW : 0 * SW + F]
                    sig_f = S[:, 1 * SW : 1 * SW + F]
                    sig_o = S[:, 2 * SW : 2 * SW + F]
                    sig_g = S[:, 3 * SW : 3 * SW + F]

                    cnew = cp.tile([128, SW], FH, tag=f"c{s}", name=f"c{s}_{t}")
                    if c_prev[s] is None:
                        # c~ = (sig(2g)-0.5)*sig_i
                        nc.vector.scalar_tensor_tensor(
                            cnew[:, 0:F], sig_g, -0.5, sig_i, OP.add, OP.mult
                        )
                    else:
                        t1 = dp.tile([128, SW], FH, tag=f"t1{s}")
                        nc.vector.scalar_tensor_tensor(
                            t1[:, 0:F], sig_g, -0.5, sig_i, OP.add, OP.mult
                        )
                        fm = dp.tile([128, SW], FH, tag=f"fm{s}")
                        nc.gpsimd.tensor_mul(fm[:, 0:F], sig_f, c_prev[s])
                        nc.vector.tensor_add(cnew[:, 0:F], t1[:, 0:F], fm[:, 0:F])
                    c_prev[s] = cnew[:, 0:F]

                    # h' = c~ * sig_o * (a_u + u*(b_u + u*cg)), u = min(c~^2, 1)
                    if t == T - 1:
                        h_new = h_final[:, c0:c1]
                    else:
                        h_new = hp.tile([128, SW], FH, tag=f"h{s}", name=f"h{s}_{t}")[
                            :, 0:F
                        ]
                    nc.vector._custom_dve(
                        tau_op,
                        out=h_new,
                        in0=cnew[:, 0:F],
                        in1=sig_o,
                        s0=ca_s[:, 0:1],
                        s1=cb_s[:, 0:1],
                        imm2=float(cg),
                    )
                    h_prev[s] = h_new

        # classifier head, per stream so it overlaps the last timesteps:
        # relu(2*W1 @ h' + b1) then W2 @ . + b2
        for s in range(NS):
            c0, c1 = CHUNK[s]
            F = c1 - c0
            hid_ps = gp.tile([64, 256], F32, tag="g", name=f"hd{s}")
            nc.tensor.matmul(
                hid_ps[:, 0:F], w1_s[:], h_final[:, c0:c1], start=True, stop=True
            )
            hr = dp.tile([64, SW], FH, tag="hr")
            nc.scalar.activation(hr[:, 0:F], hid_ps[:, 0:F], AF.Relu, bias=b1_s[:])
            sc_ps = gp.tile([2, 256], F32, tag="g", name=f"sc{s}")
            nc.tensor.matmul(sc_ps[:, 0:F], w2_s[:], hr[:, 0:F], start=True, stop=True)
            ov = dp.tile([2, SW], F32, tag="ov")
            nc.scalar.activation(ov[:, 0:F], sc_ps[:, 0:F], AF.Identity, bias=b2_s[:])
            nc.sync.dma_start(out[:, c0:c1], ov[:, 0:F])

    nc.compile()
    return nc


def _get_nc(cg: float):
    key = round(float(cg), 4)
    if key not in _NC_CACHE:
        _NC_CACHE[key] = _build_nc(key)
    return _NC_CACHE[key]


def _calibrate_tau(feats, W_ih, W_hh, bias):
    """Host-side fp32 LSTM on a batch subsample; returns per-unit deg-5
    coefficients (a_u, b_u) and global cg for p(x) ~ tanh(2x)/2 fit against
    the empirical c~ = c/2 distribution of each hidden unit."""
    sub = feats[:: max(1, feats.shape[0] // 512)][:512].astype(np.float32)
    n = sub.shape[0]
    h = np.zeros((n, H), np.float32)
    c = np.zeros((n, H), np.float32)
    samples = np.empty((T, n, H), np.float32)
    WxT = W_ih.T.astype(np.float32)
    WhT = W_hh.T.astype(np.float32)
    for t in range(T):
        g4 = sub[:, t] @ WxT + bias + h @ WhT
        i, f, gg, o = np.split(g4, 4, axis=1)
        si = 1 / (1 + np.exp(-i))
        sf = 1 / (1 + np.exp(-f))
        tg = np.tanh(gg)
        so = 1 / (1 + np.exp(-o))
        c = sf * c + si * tg
        samples[t] = c * 0.5
        h = so * np.tanh(c)

    def fit_unit(xs, r, cg=None, ngrid=1500):
        rfit = r * 1.15 + 0.02
        x = np.linspace(1e-9, rfit, ngrid)
        tgt = np.tanh(2 * x) / 2
        u = x * x
        hist, edges = np.histogram(np.abs(xs), bins=100, range=(0, rfit))
        dens = np.interp(x, 0.5 * (edges[1:] + edges[:-1]), hist.astype(np.float64))
        w = np.sqrt(dens + 0.02 * hist.max())
        if cg is None:
            A = np.stack([x, x * u, x * u * u], 1)
            co, *_ = np.linalg.lstsq(A * w[:, None], tgt * w, rcond=None)
            return co
        A = np.stack([x, x * u], 1)
        tt = tgt - cg * x * u * u
        co, *_ = np.linalg.lstsq(A * w[:, None], tt * w, rcond=None)
        return co

    r_emp = np.abs(samples).max(axis=(0, 1))
    c3 = np.array([fit_unit(samples[:, :, ui], r_emp[ui]) for ui in range(H)])
    cg = float(np.median(c3[:, 2]))
    ab = np.array([fit_unit(samples[:, :, ui], r_emp[ui], cg) for ui in range(H)])
    return ab[:, 0].astype(np.float32), ab[:, 1].astype(np.float32), cg


def _prep_weights(inputs):
    W_ih = np.asarray(inputs["W_ih"], np.float32)  # [256, 40], gate order i,f,g,o
    W_hh = np.asarray(inputs["W_hh"], np.float32)  # [256, 64]
    bias = np.asarray(inputs["b_ih"], np.float32) + np.asarray(inputs["b_hh"], np.float32)
    W1 = np.asarray(inputs["W1"], np.float32)  # [32, 64]
    b1 = np.asarray(inputs["b1"], np.float32)  # [32]
    W2 = np.asarray(inputs["W2"], np.float32)  # [1, 32]
    b2 = np.asarray(inputs["b2"], np.float32)  # [1]

    # device gate-bank order [i, f, o, g]; bank g carries 2x scale (sig(2x)
    # trick); h-side weights carry 2x for h' = h/2.
    gate_order = [0, 1, 3, 2]
    gate_scale = [1.0, 1.0, 1.0, 2.0]
    wx = np.zeros((KX, 512), _BF)
    wh = np.zeros((128, 512), _BF)
    for X, gsel in enumerate(gate_order):
        sc = gate_scale[X]
        Wxe = (sc * W_ih[64 * gsel : 64 * (gsel + 1)]).astype(np.float32)
        Whe = (2.0 * sc * W_hh[64 * gsel : 64 * (gsel + 1)]).astype(np.float32)
        be = (sc * bias[64 * gsel : 64 * (gsel + 1)]).astype(np.float32)
        wx[0:NI, 128 * X : 128 * X + 64] = Wxe.T
        wx[NI, 128 * X : 128 * X + 64] = be
        wx[NI + 1 : 2 * NI + 1, 128 * X + 64 : 128 * X + 128] = Wxe.T
        wx[2 * NI + 1, 128 * X + 64 : 128 * X + 128] = be
        wh[0:64, 128 * X : 128 * X + 64] = Whe.T
        wh[64:128, 128 * X + 64 : 128 * X + 128] = Whe.T

    w1 = np.zeros((128, 64), _BF)
    w1[0:64, 0:32] = (2.0 * W1).T
    w1[64:128, 32:64] = (2.0 * W1).T
    b1v = np.concatenate([b1, b1]).reshape(64, 1).astype(np.float32)
    w2m = np.zeros((64, 2), _BF)
    w2m[0:32, 0] = W2[0]
    w2m[32:64, 1] = W2[0]
    b2v = np.array([[b2[0]], [b2[0]]], np.float32)
    return wx, wh, w1, b1v, w2m, b2v, W_ih, W_hh, bias


def kernel(**inputs):
    global LAST_RESULT
    feats = np.asarray(inputs["feats"], np.float32)
    wx, wh, w1m, b1v, w2m, b2v, W_ih, W_hh, bias = _prep_weights(inputs)
    au, bu, cg = _calibrate_tau(feats, W_ih, W_hh, bias)
    cav = np.concatenate([au, au]).reshape(128, 1).astype(np.float32)
    cbv = np.concatenate([bu, bu]).reshape(128, 1).astype(np.float32)

    in_maps = []
    for c in range(NCORES):
        shard = feats[c * BL : (c + 1) * BL]  # [1024, 100, 40]
        x = np.ascontiguousarray(shard.transpose(1, 2, 0))  # [100, 40, 1024]
        ftc = np.empty((T, KX, HB), _BF)
        ftc[:, 0:NI, :] = x[:, :, 0:HB]
        ftc[:, NI, :] = 1.0
        ftc[:, NI + 1 : 2 * NI + 1, :] = x[:, :, HB:]
        ftc[:, 2 * NI + 1, :] = 1.0
        in_maps.append(
            {
                "ft": ftc,
                "wx": wx,
                "wh": wh,
                "w1": w1m,
                "b1": b1v,
                "w2": w2m,
                "b2": b2v,
                "ca": cav,
                "cb": cbv,
            }
        )

    nc = _get_nc(cg)
    trace = bool(os.environ.get("KERNEL_TRACE"))
    res = run_bass_kernel_spmd(nc, in_maps, core_ids=list(range(NCORES)), trace=trace)
    LAST_RESULT = res

    outs = np.empty((B, 1), np.float32)
    for c in range(NCORES):
        o = np.asarray(res.results[c]["out"])  # [2, 512]
        outs[c * BL : c * BL + HB, 0] = o[0]
        outs[c * BL + HB : (c + 1) * BL, 0] = o[1]
    return outs


if __name__ == "__main__":
    rng = np.random.default_rng(0)
    fake = {
        "feats": rng.standard_normal((B, T, NI), dtype=np.float32),
        "W_ih": rng.standard_normal((256, NI), dtype=np.float32) * 0.1,
        "W_hh": rng.standard_normal((256, H), dtype=np.float32) * 0.1,
        "b_ih": rng.standard_normal(256, dtype=np.float32) * 0.1,
        "b_hh": rng.standard_normal(256, dtype=np.float32) * 0.1,
        "W1": rng.standard_normal((32, H), dtype=np.float32) * 0.1,
        "b1": np.zeros(32, np.float32),
        "W2": rng.standard_normal((1, 32), dtype=np.float32) * 0.1,
        "b2": np.zeros(1, np.float32),
    }
    r = kernel(**fake)
    print("kernel ran, out shape", r.shape)



# revision 29
# speedup vs baseline: 1.1219x; 1.1219x over previous
"""Trainium2 Bass kernel for nn_AudioModel (LSTM(40->64) -> last-h -> MLP head).

Strategy (8 NeuronCores, pure data parallel, no collectives):
  - Each core processes a 1024-row batch shard, batch split into halves
    A/B stacked on SBUF partitions (rows 0-63 = A units, 64-127 = B units)
    so every op uses all 128 lanes; free dim = 512 batch columns per half.
  - The 512 free columns are split into NS=2 independent column streams
    (256 each) that pipeline the serial time recurrence against each
    other (anti-phased via an initial skew + PSUM round-robin).
  - The recurrence is latency-bound, so the per-step chain is minimized:
      PE:      4 h-side matmuls (K=128 block-diag) accumulate into a gate
               PSUM tile that was pre-filled ONE STEP EARLIER by the
               x-side matmuls (K=82: feats+bias for A|B block-diag).
               Hoisting the x-matmuls keeps the PE queue non-empty, which
               holds the tensor engine at its max p-state (2.4 GHz, 2x the
               mid p-state it degrades to when the queue drains each step).
      ScalarE: sigmoid in TWO bank-pair ops; bank order [i, g2, f, o] so
               the first op (PSUM bank 0) depends only on h-matmuls X=0,1
               and feeds t1 while the second sigmoid still runs.  Bank g
               is sigmoid(2a_g) so tanh(g) = 2sig-1 folds into the stt.
      VectorE: t1 = (sig_g-.5)*sig_i (stt); fm = sig_f*c~_prev;
               c~ = t1 + fm; h' = LSTM_TAU_MUL custom op
               = c~*sig_o*(a_u + u*(b_u+u*cg)), u = min(c~^2, 1) -- a
               per-partition-coefficient degree-5 odd polynomial fit to
               tanh(2c~)/2 against the empirical per-unit c~ distribution
               (host-side calibration; rel-err contribution ~3e-3).
               The whole cell stays on VectorE: in-engine issue gaps are
               ~30ns vs ~160ns cross-engine semaphore hops, and GpSimd's
               ~830ns op time would sit on the chain.
  - The cell is tracked as c~ = c/2 and h as h' = h/2 (weights pre-scaled),
    which makes every pointwise op a plain fused ALU op.
  - feats are transposed on the host into [T, 82, 512] fp16 tiles so all
    device DMAs are contiguous; biases ride constant-1 feature rows.
"""

import os
import sys
from contextlib import ExitStack, nullcontext

import numpy as np

_BF = np.dtype(np.float16)

for _p in ("/opt/trn_rl_repo",):
    if _p not in sys.path:
        sys.path.insert(0, _p)

import concourse.bass as bass
import concourse.mybir as mybir
from concourse import bacc
from concourse.bass_utils import run_bass_kernel_spmd
from concourse.tile import TileContext


def _install_ntff_hook():
    """Provide antenv.axon_hooks if the image lacks it, so trace=True works."""
    try:
        import antenv.axon_hooks  # noqa: F401

        return
    except ImportError:
        pass
    import contextlib
    import ctypes
    import types

    so_path = "/opt/axon/libaxon_pjrt.so"
    hook = None
    if os.path.exists(so_path):
        try:
            lib = ctypes.CDLL(so_path)
            if hasattr(lib, "axon_start_nrt_profile"):
                lib.axon_start_nrt_profile.argtypes = [
                    ctypes.POINTER(ctypes.c_int64),
                    ctypes.c_size_t,
                ]
                lib.axon_start_nrt_profile.restype = ctypes.c_int64
                lib.axon_stop_nrt_profile.argtypes = [ctypes.c_char_p]
                lib.axon_stop_nrt_profile.restype = ctypes.c_int64

                @contextlib.contextmanager
                def _hook(output_dir, device_ids):
                    import jax

                    jax.devices()
                    if device_ids:
                        ids = (ctypes.c_int64 * len(device_ids))(*device_ids)
                        rc = lib.axon_start_nrt_profile(ids, len(device_ids))
                    else:
                        rc = lib.axon_start_nrt_profile(None, 0)
                    if rc != 0:
                        raise RuntimeError(f"axon_start_nrt_profile rc={rc}")
                    try:
                        yield
                    finally:
                        n = lib.axon_stop_nrt_profile(str(output_dir).encode())
                        print(f"profile: {n} file(s) written to {output_dir}", file=sys.stderr)

                hook = _hook
        except OSError:
            hook = None

    mod = types.ModuleType("antenv.axon_hooks")
    mod._hook = hook
    mod.get_axon_ntff_profile_hook = lambda: mod._hook
    mod.set_axon_ntff_profile_hook = lambda h: setattr(mod, "_hook", h)
    sys.modules["antenv.axon_hooks"] = mod


_install_ntff_hook()

F32 = mybir.dt.float32
FH = mybir.dt.float16
AF = mybir.ActivationFunctionType
OP = mybir.AluOpType

B, T, NI, H = 8192, 100, 40, 64
NCORES = 8
BL = B // NCORES  # 1024 rows per core
HB = BL // 2  # 512 = half-batch (free dim of all tiles)
KX = 2 * (NI + 1)  # 82 = A feats(40) + ones(1) + B feats(40) + ones(1)
NS = 2
CHUNK = [(0, 256), (256, 512)]  # stream column ranges
SW = 256  # max stream width (tile stride)

LAST_RESULT = None
_NC_CACHE = {}
_TAU_OP = None


def _get_tau_op():
    """Register the fused tanh-cell custom DVE op (documented authoring path:
    DveOp + OPS append; uops_sha computed from lower() at registration)."""
    global _TAU_OP
    if _TAU_OP is not None:
        return _TAU_OP
    from concourse.dve_spec import Spec, Src0, Src1, C0, C1, C2, One, sq, minn, lower
    from concourse.dve_uop import DveOpSpec
    import concourse.dve_ops as dve_ops
    from concourse.dve_ops import DveOp, OPS

    name = "LSTM_TAU_MUL"
    u = minn(sq(Src0), One)

    def _ref(in0, in1, s0, s1, imm2):
        uu = np.minimum(in0.astype(np.float32) ** 2, 1.0)
        return (in0 * in1 * (s0 + uu * (s1 + uu * imm2))).astype(np.float32)

    spec = Spec(body=(Src0 * Src1) * (C0 + u * (C1 + u * C2)), reference=_ref)
    row = dve_ops._CUSTOM_DVE_ROW_BASE + len(OPS)
    shas = {
        v: DveOpSpec(name=name, opcode=row, uops=lower(spec, ver=v), rd1_en=True).sha(v)
        for v in ("v3", "v4")
    }
    op = DveOp(name, spec, subdim=False, uops_sha=shas)
    OPS.append(op)
    dve_ops._SUB_OPCODE_FOR_NAME[name] = row
    dve_ops.CUSTOM_DVE_SPECS[name] = spec
    _TAU_OP = op
    return op


def _build_nc(cg: float):
    tau_op = _get_tau_op()
    nc = bacc.Bacc("TRN2", target_bir_lowering=False, debug=False)

    ft = nc.dram_tensor("ft", [T, KX, HB], FH, kind="ExternalInput")
    wx = nc.dram_tensor("wx", [KX, 512], FH, kind="ExternalInput")
    wh = nc.dram_tensor("wh", [128, 512], FH, kind="ExternalInput")
    w1 = nc.dram_tensor("w1", [128, 64], FH, kind="ExternalInput")
    b1 = nc.dram_tensor("b1", [64, 1], F32, kind="ExternalInput")
    w2 = nc.dram_tensor("w2", [64, 2], FH, kind="ExternalInput")
    b2 = nc.dram_tensor("b2", [2, 1], F32, kind="ExternalInput")
    ca = nc.dram_tensor("ca", [128, 1], F32, kind="ExternalInput")
    cb = nc.dram_tensor("cb", [128, 1], F32, kind="ExternalInput")
    out = nc.dram_tensor("out", [2, HB], F32, kind="ExternalOutput")

    with TileContext(nc) as tc, ExitStack() as ctx:
        const = ctx.enter_context(tc.tile_pool(name="const", bufs=1))
        ftp = ctx.enter_context(tc.tile_pool(name="ftp", bufs=16))
        gp = ctx.enter_context(tc.tile_pool(name="gp", bufs=4, space="PSUM"))
        sp = ctx.enter_context(tc.tile_pool(name="sp", bufs=4))
        dp = ctx.enter_context(tc.tile_pool(name="dp", bufs=4))
        cp = ctx.enter_context(tc.tile_pool(name="cp", bufs=3))
        hp = ctx.enter_context(tc.tile_pool(name="hp", bufs=4))

        # Only ft0 + wx load BEFORE the step-0 x-matmuls: the first ldweights
        # waits on the DMA completion counter in emission order, so every DMA
        # emitted before it delays pipeline start.  Everything else (wh,
        # classifier consts, ft1/ft2) is emitted after the x(0) block and
        # loads while step 0 runs.
        ft0_t = ftp.tile([KX, HB], FH, tag="ft", name="ft0")
        nc.sync.dma_start(ft0_t[:], ft[0])
        wx_s = const.tile([KX, 512], FH)
        nc.sync.dma_start(wx_s[:], wx[:, :])

        wh_s = const.tile([128, 512], FH)
        w1_s = const.tile([128, 64], FH)
        b1_s = const.tile([64, 1], F32)
        w2_s = const.tile([64, 2], FH)
        b2_s = const.tile([2, 1], F32)
        ca_s = const.tile([128, 1], F32)
        cb_s = const.tile([128, 1], F32)

        h_final = const.tile([128, HB], FH)

        h_prev = [None] * NS
        c_prev = [None] * NS

        # ft prefetch: x-matmuls for step t+1 are emitted during step t, so
        # keep 3 tiles in flight.
        ft_tiles = {0: ft0_t}

        def load_ft(t):
            if t < T:
                ft_t = ftp.tile([KX, HB], FH, tag="ft", name=f"ft{t}")
                nc.sync.dma_start(ft_t[:], ft[t])
                ft_tiles[t] = ft_t

        def load_consts():
            nc.scalar.dma_start(wh_s[:], wh[:, :])
            nc.scalar.dma_start(ca_s[:], ca[:, :])
            nc.scalar.dma_start(cb_s[:], cb[:, :])
            nc.scalar.dma_start(w1_s[:], w1[:, :])
            nc.scalar.dma_start(b1_s[:], b1[:, :])
            nc.scalar.dma_start(w2_s[:], w2[:, :])
            nc.scalar.dma_start(b2_s[:], b2[:, :])

        def x_mms(t, s, stop):
            """x-side matmuls for step t, stream s, into a fresh gate tile.

            gate PSUM tile: 4 gates at 256-f32 (1KB) stride = 2 banks;
            gates 0,1 share bank 0 and gates 2,3 share bank 1.  start=True
            clears has_written for the WHOLE bank, so only the first matmul
            into each bank may set it; the second gate's x-matmul runs with
            flags=0, which overwrites where the bit is unset (its region)
            and the h-matmuls then accumulate where it is set.
            """
            c0, c1 = CHUNK[s]
            F = c1 - c0
            g = gp.tile([128, 4 * 256], F32, tag="g", name=f"g{s}_{t}")
            ft_t = ft_tiles[t]
            for X in range(4):
                nc.tensor.matmul(
                    g[:, 256 * X : 256 * X + F],
                    wx_s[:, 128 * X : 128 * (X + 1)],
                    ft_t[:, c0:c1],
                    start=(X % 2 == 0),
                    stop=stop,
                    skip_group_check=(X % 2 == 1),
                )
            return g

        # step-0 gates are x-side only (h0 = 0)
        g_cur = [x_mms(0, s, stop=True) for s in range(NS)]

        # remaining loads start only now, behind the x(0) block in the DMA
        # completion counter order
        load_consts()
        for t0 in range(1, 3):
            load_ft(t0)

        for t in range(T):
            load_ft(t + 3)
            g_next = [None] * NS

            for s in range(NS):
                c0, c1 = CHUNK[s]
                F = c1 - c0
                skew = tc.tile_wait_until(0.0017 * s, enable=(t == 0)) if t == 0 else nullcontext()
                with skew:
                    g = g_cur[s]
                    if h_prev[s] is not None:
                        for X in range(4):
                            nc.tensor.matmul(
                                g[:, 256 * X : 256 * X + F],
                                wh_s[:, 128 * X : 128 * (X + 1)],
                                h_prev[s],
                                start=False,
                                stop=True,
                                skip_group_check=(X % 2 == 1),
                            )
                    # hoist next step's x-side matmuls: they have no deps on
                    # the recurrence, so they keep the PE busy (and ramped to
                    # max p-state) while this step's pointwise chain runs.
                    if t + 1 < T:
                        g_next[s] = x_mms(t + 1, s, stop=False)

                    # sigmoid in two bank-pair ops; bank order [i, g2, f, o].
                    # The first op covers PSUM bank 0 (gates i, g2) and only
                    # depends on h-matmuls X=0,1, so t1 starts earlier.
                    S = sp.tile([128, 4 * SW], FH, tag=f"S{s}")
                    nc.scalar.activation(S[:, 0 : 2 * SW], g[:, 0:512], AF.Sigmoid)
                    nc.scalar.activation(S[:, 2 * SW : 4 * SW], g[:, 512:1024], AF.Sigmoid)
                    sig_i = S[:, 0 * SW : 0 * SW + F]
                    sig_g = S[:, 1 * SW : 1 * SW + F]
                    sig_f = S[:, 2 * SW : 2 * SW + F]
                    sig_o = S[:, 3 * SW : 3 * SW + F]

                    cnew = cp.tile([128, SW], FH, tag=f"c{s}", name=f"c{s}_{t}")
                    if c_prev[s] is None:
                        # c~ = (sig(2g)-0.5)*sig_i
                        nc.vector.scalar_tensor_tensor(
                            cnew[:, 0:F], sig_g, -0.5, sig_i, OP.add, OP.mult
                        )
                    else:
                        # whole cell on DVE back-to-back: in-engine gaps are
                        # ~30ns vs ~160ns cross-engine semaphore hops, and the
                        # Pool engine's ~830ns op time was on the chain.
                        t1 = dp.tile([128, SW], FH, tag=f"t1{s}")
                        nc.vector.scalar_tensor_tensor(
                            t1[:, 0:F], sig_g, -0.5, sig_i, OP.add, OP.mult
                        )
                        fm = dp.tile([128, SW], FH, tag=f"fm{s}")
                        nc.vector.tensor_mul(fm[:, 0:F], sig_f, c_prev[s])
                        nc.vector.tensor_add(cnew[:, 0:F], t1[:, 0:F], fm[:, 0:F])
                    c_prev[s] = cnew[:, 0:F]

                    # h' = c~ * sig_o * (a_u + u*(b_u + u*cg)), u = min(c~^2, 1)
                    if t == T - 1:
                        h_new = h_final[:, c0:c1]
                    else:
                        h_new = hp.tile([128, SW], FH, tag=f"h{s}", name=f"h{s}_{t}")[
                            :, 0:F
                        ]
                    nc.vector._custom_dve(
                        tau_op,
                        out=h_new,
                        in0=cnew[:, 0:F],
                        in1=sig_o,
                        s0=ca_s[:, 0:1],
                        s1=cb_s[:, 0:1],
                        imm2=float(cg),
                    )
                    h_prev[s] = h_new

            g_cur = g_next

        # classifier head, per stream so it overlaps the last timesteps:
        # relu(2*W1 @ h' + b1) then W2 @ . + b2
        for s in range(NS):
            c0, c1 = CHUNK[s]
            F = c1 - c0
            hid_ps = gp.tile([64, 256], F32, tag="g", name=f"hd{s}")
            nc.tensor.matmul(
                hid_ps[:, 0:F], w1_s[:], h_final[:, c0:c1], start=True, stop=True
            )
            hr = dp.tile([64, SW], FH, tag="hr")
            nc.scalar.activation(hr[:, 0:F], hid_ps[:, 0:F], AF.Relu, bias=b1_s[:])
            sc_ps = gp.tile([2, 256], F32, tag="g", name=f"sc{s}")
            nc.tensor.matmul(sc_ps[:, 0:F], w2_s[:], hr[:, 0:F], start=True, stop=True)
            ov = dp.tile([2, SW], F32, tag="ov")
            nc.scalar.activation(ov[:, 0:F], sc_ps[:, 0:F], AF.Identity, bias=b2_s[:])
            nc.sync.dma_start(out[:, c0:c1], ov[:, 0:F])

    nc.compile()
    return nc


def _get_nc(cg: float):
    key = round(float(cg), 4)
    if key not in _NC_CACHE:
        _NC_CACHE[key] = _build_nc(key)
    return _NC_CACHE[key]


def _calibrate_tau(feats, W_ih, W_hh, bias):
    """Host-side fp32 LSTM on a batch subsample; returns per-unit deg-5
    coefficients (a_u, b_u) and global cg for p(x) ~ tanh(2x)/2 fit against
    the empirical c~ = c/2 distribution of each hidden unit."""
    sub = feats[:: max(1, feats.shape[0] // 512)][:512].astype(np.float32)
    n = sub.shape[0]
    h = np.zeros((n, H), np.float32)
    c = np.zeros((n, H), np.float32)
    samples = np.empty((T, n, H), np.float32)
    WxT = W_ih.T.astype(np.float32)
    WhT = W_hh.T.astype(np.float32)
    for t in range(T):
        g4 = sub[:, t] @ WxT + bias + h @ WhT
        i, f, gg, o = np.split(g4, 4, axis=1)
        si = 1 / (1 + np.exp(-i))
        sf = 1 / (1 + np.exp(-f))
        tg = np.tanh(gg)
        so = 1 / (1 + np.exp(-o))
        c = sf * c + si * tg
        samples[t] = c * 0.5
        h = so * np.tanh(c)

    def fit_unit(xs, r, cg=None, ngrid=1500):
        rfit = r * 1.15 + 0.02
        x = np.linspace(1e-9, rfit, ngrid)
        tgt = np.tanh(2 * x) / 2
        u = x * x
        hist, edges = np.histogram(np.abs(xs), bins=100, range=(0, rfit))
        dens = np.interp(x, 0.5 * (edges[1:] + edges[:-1]), hist.astype(np.float64))
        w = np.sqrt(dens + 0.02 * hist.max())
        if cg is None:
            A = np.stack([x, x * u, x * u * u], 1)
            co, *_ = np.linalg.lstsq(A * w[:, None], tgt * w, rcond=None)
            return co
        A = np.stack([x, x * u], 1)
        tt = tgt - cg * x * u * u
        co, *_ = np.linalg.lstsq(A * w[:, None], tt * w, rcond=None)
        return co

    r_emp = np.abs(samples).max(axis=(0, 1))
    c3 = np.array([fit_unit(samples[:, :, ui], r_emp[ui]) for ui in range(H)])
    cg = float(np.median(c3[:, 2]))
    ab = np.array([fit_unit(samples[:, :, ui], r_emp[ui], cg) for ui in range(H)])
    return ab[:, 0].astype(np.float32), ab[:, 1].astype(np.float32), cg


def _prep_weights(inputs):
    W_ih = np.asarray(inputs["W_ih"], np.float32)  # [256, 40], gate order i,f,g,o
    W_hh = np.asarray(inputs["W_hh"], np.float32)  # [256, 64]
    bias = np.asarray(inputs["b_ih"], np.float32) + np.asarray(inputs["b_hh"], np.float32)
    W1 = np.asarray(inputs["W1"], np.float32)  # [32, 64]
    b1 = np.asarray(inputs["b1"], np.float32)  # [32]
    W2 = np.asarray(inputs["W2"], np.float32)  # [1, 32]
    b2 = np.asarray(inputs["b2"], np.float32)  # [1]

    # device gate-bank order [i, g, f, o]; bank g carries 2x scale (sig(2x)
    # trick); h-side weights carry 2x for h' = h/2.  Banks i,g share PSUM
    # bank 0 so the first sigmoid op (which feeds t1) only depends on the
    # first two h-matmuls.
    gate_order = [0, 2, 1, 3]
    gate_scale = [1.0, 2.0, 1.0, 1.0]
    wx = np.zeros((KX, 512), _BF)
    wh = np.zeros((128, 512), _BF)
    for X, gsel in enumerate(gate_order):
        sc = gate_scale[X]
        Wxe = (sc * W_ih[64 * gsel : 64 * (gsel + 1)]).astype(np.float32)
        Whe = (2.0 * sc * W_hh[64 * gsel : 64 * (gsel + 1)]).astype(np.float32)
        be = (sc * bias[64 * gsel : 64 * (gsel + 1)]).astype(np.float32)
        wx[0:NI, 128 * X : 128 * X + 64] = Wxe.T
        wx[NI, 128 * X : 128 * X + 64] = be
        wx[NI + 1 : 2 * NI + 1, 128 * X + 64 : 128 * X + 128] = Wxe.T
        wx[2 * NI + 1, 128 * X + 64 : 128 * X + 128] = be
        wh[0:64, 128 * X : 128 * X + 64] = Whe.T
        wh[64:128, 128 * X + 64 : 128 * X + 128] = Whe.T

    w1 = np.zeros((128, 64), _BF)
    w1[0:64, 0:32] = (2.0 * W1).T
    w1[64:128, 32:64] = (2.0 * W1).T
    b1v = np.concatenate([b1, b1]).reshape(64, 1).astype(np.float32)
    w2m = np.zeros((64, 2), _BF)
    w2m[0:32, 0] = W2[0]
    w2m[32:64, 1] = W2[0]
    b2v = np.array([[b2[0]], [b2[0]]], np.float32)
    return wx, wh, w1, b1v, w2m, b2v, W_ih, W_hh, bias


def kernel(**inputs):
    global LAST_RESULT
    feats = np.asarray(inputs["feats"], np.float32)
    wx, wh, w1m, b1v, w2m, b2v, W_ih, W_hh, bias = _prep_weights(inputs)
    au, bu, cg = _calibrate_tau(feats, W_ih, W_hh, bias)
    cav = np.concatenate([au, au]).reshape(128, 1).astype(np.float32)
    cbv = np.concatenate([bu, bu]).reshape(128, 1).astype(np.float32)

    in_maps = []
    for c in range(NCORES):
        shard = feats[c * BL : (c + 1) * BL]  # [1024, 100, 40]
        x = np.ascontiguousarray(shard.transpose(1, 2, 0))  # [100, 40, 1024]
        ftc = np.empty((T, KX, HB), _BF)
        ftc[:, 0:NI, :] = x[:, :, 0:HB]
        ftc[:, NI, :] = 1.0
        ftc[:, NI + 1 : 2 * NI + 1, :] = x[:, :, HB:]
        ftc[:, 2 * NI + 1, :] = 1.0
        in_maps.append(
            {
                "ft": ftc,
                "wx": wx,
                "wh": wh,
                "w1": w1m,
                "b1": b1v,
                "w2": w2m,
                "b2": b2v,
                "ca": cav,
                "cb": cbv,
            }
        )

    nc = _get_nc(cg)
    trace = bool(os.environ.get("KERNEL_TRACE"))
    res = run_bass_kernel_spmd(nc, in_maps, core_ids=list(range(NCORES)), trace=trace)
    LAST_RESULT = res

    outs = np.empty((B, 1), np.float32)
    for c in range(NCORES):
        o = np.asarray(res.results[c]["out"])  # [2, 512]
        outs[c * BL : c * BL + HB, 0] = o[0]
        outs[c * BL + HB : (c + 1) * BL, 0] = o[1]
    return outs


if __name__ == "__main__":
    rng = np.random.default_rng(0)
    fake = {
        "feats": rng.standard_normal((B, T, NI), dtype=np.float32),
        "W_ih": rng.standard_normal((256, NI), dtype=np.float32) * 0.1,
        "W_hh": rng.standard_normal((256, H), dtype=np.float32) * 0.1,
        "b_ih": rng.standard_normal(256, dtype=np.float32) * 0.1,
        "b_hh": rng.standard_normal(256, dtype=np.float32) * 0.1,
        "W1": rng.standard_normal((32, H), dtype=np.float32) * 0.1,
        "b1": np.zeros(32, np.float32),
        "W2": rng.standard_normal((1, 32), dtype=np.float32) * 0.1,
        "b2": np.zeros(1, np.float32),
    }
    r = kernel(**fake)
    print("kernel ran, out shape", r.shape)

